# revision 38
# baseline (speedup 1.0000x reference)
"""MultiHeadAttention Trainium2 kernel.

Sharding: B=2 batches x H=16 heads = 32 (b,h) pairs -> 4 heads per core.
Cores 0-3 handle batch 0 (heads 4c..4c+3), cores 4-7 batch 1.
Each core computes q/k/v projections for its head slice, transposed-scores
attention, and a partial output projection (sum over its heads of
o_h @ Wo[h-slice]).  Host sums the 4 bf16 partials per batch and adds bo.

All-bf16 datapath (fp8 operand storage was measured to break the 2e-2
tolerance: qk8 5.5e-2, P8+v8+o8 4.2e-2).  Biases fold into the PSUM->SBUF
copies (per-partition tensor_scalar_add for q/k, broadcast add for v), so
the PE runs no bias rows.  Softmax skips max-subtraction (scores ~ N(0,1));
the mask multiply runs on DVE (bf16 2x) against a per-group streamed maskT;
denominators come from a ones-column appended to V; and 1/denominator is
folded into the PSUM->SBUF copy of o^T.

Schedule: the attention c-loop is Act-bound (one [128,1024] exp per chunk,
1038ns each), so every other engine hides under it.  The loop runs per
(sq-group, head-pair) pass so the P@V accumulators need only 2 PSUM banks,
leaving a dedicated 2-bank projection pool; projection and output-
projection matmuls drip into the loop a few per chunk from a deadline-
sorted worklist, keeping the PE dense without stalling the exp-paced
scores-PSUM rotation (2 x [128,1024]).  P@V trails the exp/mask pipeline
by 4 chunks to decouple DVE jitter.  A dummy-matmul chain warms the PE
p-state during the initial DMA fill; the first k/q projections stream in
half-tensor DMA pieces; the tail reorders the last P@V half-pairs and
splits the final normalize per 128 columns so the last output projections
start as early as possible.
"""

import sys

sys.path.insert(0, '/opt/trn_rl_repo')

import numpy as np

B, S, D = 2, 2048, 1024
H = 16
DK = 64
HC = 4            # heads per core
NC_ = HC * DK     # 256 projected dims per core
NT = NC_ // 128   # head-pair tiles per core
NCORES = 8

_cached = {}


def _build_nc():
    import concourse.bacc as bacc
    import concourse.mybir as mybir
    from concourse.tile import TileContext

    f32 = mybir.dt.float32
    bf16 = mybir.dt.bfloat16
    Exp = mybir.ActivationFunctionType.Exp
    Identity = mybir.ActivationFunctionType.Identity

    nc = bacc.Bacc()

    XQT = nc.declare_dram_parameter("xqT", [D, S], bf16, isOutput=False)
    XKT = nc.declare_dram_parameter("xkT", [D, S], bf16, isOutput=False)
    XVT = nc.declare_dram_parameter("xvT", [D, S], bf16, isOutput=False)
    WQ = nc.declare_dram_parameter("wq", [D, NC_], bf16, isOutput=False)
    WK = nc.declare_dram_parameter("wk", [D, NC_], bf16, isOutput=False)
    WV = nc.declare_dram_parameter("wv", [D, NC_], bf16, isOutput=False)
    WO = nc.declare_dram_parameter("wo", [NC_, D], bf16, isOutput=False)
    BQC = nc.declare_dram_parameter("bqc", [128, 2], f32, isOutput=False)
    BKC = nc.declare_dram_parameter("bkc", [128, 2], f32, isOutput=False)
    BV = nc.declare_dram_parameter("bv", [1, NC_], f32, isOutput=False)
    MT = nc.declare_dram_parameter("maskT", [S, S], bf16, isOutput=False)
    OUT = nc.declare_dram_parameter("out", [S, D], bf16, isOutput=True)

    NDC = D // 128           # 8 d chunks
    NG = S // 512            # 4 sq groups / k s-groups
    NCk = S // 128           # 16 sk chunks

    with TileContext(nc) as tc:
        import contextlib
        ctx = contextlib.ExitStack()
        with ctx:
            consts = ctx.enter_context(tc.tile_pool(name="consts", bufs=1))
            xts = ctx.enter_context(tc.tile_pool(name="xts", bufs=1))
            pts = ctx.enter_context(tc.tile_pool(name="pts", bufs=3))
            smalls = ctx.enter_context(tc.tile_pool(name="smalls", bufs=3))
            outs = ctx.enter_context(tc.tile_pool(name="outs", bufs=3))
            mts = ctx.enter_context(tc.tile_pool(name="mts", bufs=2))
            sp = ctx.enter_context(tc.tile_pool(name="sp", bufs=2, space="PSUM"))
            pp = ctx.enter_context(tc.tile_pool(name="pp", bufs=2, space="PSUM"))
            op = ctx.enter_context(tc.tile_pool(name="op", bufs=1, space="PSUM"))

            # ---- constants (DMA order matters: first-needed first) ----
            wk_sb = consts.tile([128, NDC, NC_], bf16)
            wq_sb = consts.tile([128, NDC, NC_], bf16)
            wv_sb = consts.tile([128, NDC, NC_], bf16)
            wo_sb = consts.tile([128, NT, D], bf16)
            bqc_sb = consts.tile([128, NT], f32)
            bkc_sb = consts.tile([128, NT], f32)
            bv_row = consts.tile([1, NC_], f32)

            xtiles = {}

            def emit_x_dma(which, g, bufs=4, split=False):
                X = {"k": XKT, "q": XQT, "v": XVT}[which]
                xg = xts.tile([128, NDC, 512], bf16, tag=f"x{which}",
                              name=f"x{which}{g}", bufs=bufs)
                if split:
                    for hh in range(2):
                        nc.sync.dma_start(
                            out=xg[:, 4 * hh:4 * (hh + 1), :],
                            in_=X[4 * hh * 128:4 * (hh + 1) * 128,
                                  g * 512:(g + 1) * 512]
                            .rearrange("(c p) n -> p c n", p=128))
                else:
                    nc.sync.dma_start(
                        out=xg,
                        in_=X[:, g * 512:(g + 1) * 512].rearrange("(c p) n -> p c n", p=128))
                xtiles[(which, g)] = xg

            mtiles = {}

            def emit_mask_dma(g, piece=None, c0=None, c1=None):
                # mask columns for sq-group g; [128, NCk, 512] per group.
                if g not in mtiles:
                    mtiles[g] = mts.tile([128, NCk, 512], bf16, tag="mt",
                                         name=f"mt{g}", bufs=2)
                if c0 is None:
                    if piece is None:
                        c0, c1 = 0, NCk
                    else:
                        c0, c1 = 4 * piece, 4 * piece + 4
                nc.sync.dma_start(
                    out=mtiles[g][:, c0:c1, :],
                    in_=MT[c0 * 128:c1 * 128, g * 512:(g + 1) * 512]
                    .rearrange("(c p) s -> p c s", p=128))

            warm = consts.tile([1, 512], bf16)
            nc.vector.memset(warm, 1.0)
            for wu in range(2):
                wps = pp.tile([128, 512], f32, tag="pp", name=f"warmps{wu}")
                for _ in range(3):
                    nc.tensor.matmul(wps[0:1, :], warm[0:1, 0:1], warm[0:1, :],
                                     start=True, stop=True)

            xk0 = xts.tile([128, NDC, 512], bf16, tag="xk", name="xk0", bufs=4)
            xq0 = xts.tile([128, NDC, 512], bf16, tag="xq", name="xq0", bufs=2)
            xtiles[("k", 0)] = xk0
            xtiles[("q", 0)] = xq0
            nc.sync.dma_start(out=wk_sb[:, 0:4, :],
                              in_=WK[0:512].rearrange("(c p) n -> p c n", p=128))
            nc.sync.dma_start(out=xk0[:, 0:4, :],
                              in_=XKT[0:512, 0:512].rearrange("(c p) n -> p c n", p=128))
            nc.sync.dma_start(out=wq_sb[:, 0:4, :],
                              in_=WQ[0:512].rearrange("(c p) n -> p c n", p=128))
            nc.sync.dma_start(out=xq0[:, 0:4, :],
                              in_=XQT[0:512, 0:512].rearrange("(c p) n -> p c n", p=128))
            nc.sync.dma_start(out=wk_sb[:, 4:8, :],
                              in_=WK[512:1024].rearrange("(c p) n -> p c n", p=128))
            nc.sync.dma_start(out=xk0[:, 4:8, :],
                              in_=XKT[512:1024, 0:512].rearrange("(c p) n -> p c n", p=128))
            nc.sync.dma_start(out=bkc_sb, in_=BKC[:])
            nc.sync.dma_start(out=wq_sb[:, 4:8, :],
                              in_=WQ[512:1024].rearrange("(c p) n -> p c n", p=128))
            nc.sync.dma_start(out=xq0[:, 4:8, :],
                              in_=XQT[512:1024, 0:512].rearrange("(c p) n -> p c n", p=128))
            nc.sync.dma_start(out=bqc_sb, in_=BQC[:])
            nc.sync.dma_start(out=wv_sb, in_=WV[:].rearrange("(c p) n -> p c n", p=128))
            emit_x_dma("v", 0, bufs=3)
            emit_mask_dma(0, c0=0, c1=2)
            nc.sync.dma_start(out=bv_row, in_=BV[:])
            emit_mask_dma(0, c0=2, c1=4)
            emit_x_dma("k", 1, split=True)
            emit_mask_dma(0, c0=4, c1=6)
            emit_mask_dma(0, c0=6, c1=8)
            emit_x_dma("v", 1, bufs=3, split=True)
            emit_mask_dma(0, c0=8, c1=10)
            emit_mask_dma(0, c0=10, c1=12)
            emit_x_dma("q", 1, bufs=2)
            emit_mask_dma(0, c0=12, c1=14)
            emit_x_dma("k", 2)
            emit_mask_dma(0, c0=14, c1=16)
            emit_x_dma("v", 2, bufs=3)
            emit_x_dma("k", 3)
            emit_x_dma("v", 3, bufs=3)
            nc.sync.dma_start(out=wo_sb, in_=WO[:].rearrange("(c p) n -> p c n", p=128))
            bv_bc = consts.tile([128, NC_], f32)
            nc.gpsimd.partition_broadcast(bv_bc, bv_row)

            qT = [consts.tile([128, S], bf16, tag=f"qT{i}", name=f"qT{i}") for i in range(NT)]
            kT = [consts.tile([128, S], bf16, tag=f"kT{i}", name=f"kT{i}") for i in range(NT)]
            v_aug = consts.tile([128, NCk, HC * 65], bf16)
            nc.gpsimd.memset(v_aug, 1.0)
            oTn = [consts.tile([128, S], bf16, tag=f"oTn{i}", name=f"oTn{i}") for i in range(NT)]

            # ---- worklist quanta (deadline, closure) ----
            def kq_nt_quanta(g, which, nt, dl):
                """4 quanta of one n-tile of a k/q projection, deadlines dl-3..dl."""
                W, BC, T = ((wk_sb, bkc_sb, kT) if which == "k"
                            else (wq_sb, bqc_sb, qT))
                state = {}

                def start(state=state, nt=nt, g=g, which=which):
                    state["ps"] = pp.tile([128, 512], f32, tag="pp",
                                          name=f"{which}ps{g}_{nt}")

                def mms(dc, state=state, nt=nt, g=g, W=W, which=which):
                    xg = xtiles[(which, g)]
                    nc.tensor.matmul(
                        state["ps"][:],
                        W[:, dc, nt * 128:(nt + 1) * 128],
                        xg[:, dc, :],
                        start=(dc == 0), stop=(dc == NDC - 1),
                    )

                def fin(state=state, nt=nt, g=g, T=T, BC=BC):
                    nc.vector.tensor_scalar_add(
                        T[nt][:, g * 512:(g + 1) * 512], state["ps"][:],
                        BC[:, nt:nt + 1])

                return [(dl - 3, lambda s=start, m=mms: (s(), m(0), m(1))),
                        (dl - 2, lambda m=mms: (m(2), m(3))),
                        (dl - 1, lambda m=mms: (m(4), m(5))),
                        (dl, lambda m=mms, f=fin: (m(6), m(7), f()))]

            def v_sl_quanta(g, sl, dl):
                st = 4 * g + sl
                state = {}

                def start(state=state, st=st):
                    state["ps"] = pp.tile([128, 512], f32, tag="pp", name=f"vps{st}")

                def mms(dc0, state=state, sl=sl, g=g):
                    xgv = xtiles[("v", g)]
                    for dc in range(dc0, dc0 + 4):
                        nc.tensor.matmul(
                            state["ps"][:, 0:NC_],
                            xgv[:, dc, sl * 128:(sl + 1) * 128],
                            wv_sb[:, dc, :],
                            start=(dc == 0), stop=(dc == NDC - 1),
                        )

                def fin(state=state, st=st):
                    for h in range(HC):
                        nc.vector.tensor_add(
                            out=v_aug[:, st, h * 65:h * 65 + 64],
                            in0=state["ps"][:, h * 64:(h + 1) * 64],
                            in1=bv_bc[:, h * 64:(h + 1) * 64],
                        )

                return [(dl - 1, lambda s=start, m=mms: (s(), m(0))),
                        (dl, lambda m=mms, f=fin: (m(4), f()))]

            def kq_nt_quanta_fine(g, which, nt, dl0):
                """8 single-matmul quanta (contiguous deadlines) + fin."""
                W, BC, T = ((wk_sb, bkc_sb, kT) if which == "k"
                            else (wq_sb, bqc_sb, qT))
                state = {}

                def mm(dc, state=state, nt=nt, g=g, W=W, which=which):
                    if dc == 0:
                        state["ps"] = pp.tile([128, 512], f32, tag="pp",
                                              name=f"{which}ps{g}_{nt}")
                    xg = xtiles[(which, g)]
                    nc.tensor.matmul(
                        state["ps"][:],
                        W[:, dc, nt * 128:(nt + 1) * 128],
                        xg[:, dc, :],
                        start=(dc == 0), stop=(dc == NDC - 1),
                    )

                def fin(state=state, nt=nt, g=g, T=T, BC=BC):
                    nc.vector.tensor_scalar_add(
                        T[nt][:, g * 512:(g + 1) * 512], state["ps"][:],
                        BC[:, nt:nt + 1])

                out = [(dl0 + j, lambda mm=mm, j=j: mm(j)) for j in range(NDC - 1)]
                out.append((dl0 + NDC - 1, lambda mm=mm, f=fin: (mm(NDC - 1), f())))
                return out

            def outproj_quanta_fine(st, dl):
                """4 single-matmul quanta + two copy/DMA finishers."""
                state = {}
                out = []
                for j, (dg, kc) in enumerate([(0, 0), (0, 1), (1, 0), (1, 1)]):
                    def mk(dg=dg, kc=kc, state=state, st=st):
                        if kc == 0:
                            state[dg] = pp.tile([128, 512], f32, tag="pp",
                                                name=f"fps{st}_{dg}")
                        nc.tensor.matmul(
                            state[dg][:],
                            oTn[kc][:, st * 128:(st + 1) * 128],
                            wo_sb[:, kc, dg * 512:(dg + 1) * 512],
                            start=(kc == 0), stop=(kc == NT - 1),
                        )
                    out.append((dl + j, mk))

                def fin0(state=state, st=st):
                    osb = outs.tile([128, D], bf16)
                    state["osb"] = osb
                    nc.vector.tensor_copy(osb[:, 0:512], state[0][:])

                def fin1(state=state, st=st):
                    osb = state["osb"]
                    nc.vector.tensor_copy(osb[:, 512:1024], state[1][:])
                    nc.sync.dma_start(out=OUT[st * 128:(st + 1) * 128, :], in_=osb[:])
                out.append((dl + 4, fin0))
                out.append((dl + 5, fin1))
                return out

            def outproj_quanta(st, dl):
                state = {}
                out = []
                for dg in range(D // 512):
                    def mk(dg=dg, state=state, st=st):
                        ps = pp.tile([128, 512], f32, tag="pp", name=f"fps{st}_{dg}")
                        state[dg] = ps
                        for kc in range(NT):
                            nc.tensor.matmul(
                                ps[:],
                                oTn[kc][:, st * 128:(st + 1) * 128],
                                wo_sb[:, kc, dg * 512:(dg + 1) * 512],
                                start=(kc == 0), stop=(kc == NT - 1),
                            )
                    out.append((dl + dg, mk))

                def fin0(state=state, st=st):
                    osb = outs.tile([128, D], bf16)
                    state["osb"] = osb
                    nc.vector.tensor_copy(osb[:, 0:512], state[0][:])

                def fin1(state=state, st=st):
                    osb = state["osb"]
                    nc.vector.tensor_copy(osb[:, 512:1024], state[1][:])
                    nc.sync.dma_start(out=OUT[st * 128:(st + 1) * 128, :], in_=osb[:])
                out.append((dl + 2, fin0))
                out.append((dl + 3, fin1))
                return out

            # ---- prologue: just the first head-pair's k/q projections ----
            for _, q in kq_nt_quanta(0, "k", 0, 0):
                q()
            for _, q in kq_nt_quanta(0, "q", 0, 0):
                q()

            # ---- attention: per (sq-group, head-pair) pass ----
            for g in range(NG):
                if g + 1 < NG:
                    emit_mask_dma(g + 1)
                work = []
                if g == 0:
                    work += kq_nt_quanta(0, "q", 1, 0)
                    work += kq_nt_quanta(0, "k", 1, 1)
                    for sl in range(4):
                        work += v_sl_quanta(0, sl, sl + 2)
                    for s2 in range(1, NG):
                        work += kq_nt_quanta(s2, "k", 0, 4 * s2 - 2)
                        for sl in range(4):
                            work += v_sl_quanta(s2, sl, 4 * s2 + sl)
                    for s2 in range(1, NG):
                        work += kq_nt_quanta(s2, "k", 1, 14 + 4 * s2)
                    work += kq_nt_quanta(1, "q", 0, 26)
                    work += kq_nt_quanta(1, "q", 1, 30)
                else:
                    if g < NG - 1:
                        emit_x_dma("q", g + 1, bufs=2)
                        work += kq_nt_quanta_fine(g + 1, "q", 0, 0)
                        work += kq_nt_quanta_fine(g + 1, "q", 1, 8)
                    sts = {1: [0, 1], 2: [2, 3, 4, 5], 3: [6, 7, 8, 9, 10, 11]}[g]
                    base = 16 if g < NG - 1 else 0
                    span = max(1, (31 - base) // len(sts))
                    for i, st in enumerate(sts):
                        work += outproj_quanta_fine(st, base + span * i)
                work.sort(key=lambda x: x[0])
                wi = 0

                def drip(t, work=work):
                    nonlocal wi
                    rem = len(work) - wi
                    if rem <= 0:
                        return
                    iters_left = max(1, 32 - t)
                    n = max(0, -(-rem // iters_left))
                    while wi < len(work) and (work[wi][0] <= t + 2 or n > 0):
                        work[wi][1]()
                        wi += 1
                        n -= 1

                for p in range(2):
                    ot = [op.tile([65, 512], f32, tag=f"ot{half}",
                                  name=f"ot{p}_{half}_{g}")
                          for half in range(2)]

                    def emit_v(c, pt, p=p, ot=ot):
                        for half in range(2):
                            h = 2 * p + half
                            nc.tensor.matmul(
                                ot[half][:],
                                v_aug[:, c, h * 65:(h + 1) * 65],
                                pt[:, half * 512:(half + 1) * 512],
                                start=(c == 0), stop=(c == NCk - 1),
                            )

                    ptq = []
                    for c in range(NCk):
                        sps = sp.tile([128, 1024], f32, tag="sps",
                                      name=f"sps{g}_{p}_{c}")
                        for half in range(2):
                            nc.tensor.matmul(
                                sps[:, half * 512:(half + 1) * 512],
                                kT[p][half * 64:half * 64 + 64, c * 128:(c + 1) * 128],
                                qT[p][half * 64:half * 64 + 64, g * 512:(g + 1) * 512],
                                start=True, stop=True,
                            )
                        pt = pts.tile([128, 1024], bf16, tag="pt",
                                      name=f"pt{g}_{p}_{c}", bufs=6)
                        nc.scalar.activation(pt[:], sps[:], Exp, scale=0.125)
                        for half in range(2):
                            nc.vector.tensor_mul(
                                pt[:, half * 512:(half + 1) * 512],
                                pt[:, half * 512:(half + 1) * 512],
                                mtiles[g][:, c, :])
                        ptq.append(pt)
                        if len(ptq) > 4:
                            emit_v(c - 4, ptq.pop(0))
                        drip(p * NCk + c)
                    if g == NG - 1 and p == 1:
                        # half-major: all h2 PVs then all h3, so the h2
                        # denominator reciprocal starts earlier
                        tail_pts = list(ptq)
                        for half in range(2):
                            for i, cc in enumerate(range(NCk - 4, NCk)):
                                h = 2 * p + half
                                nc.tensor.matmul(
                                    ot[half][:],
                                    v_aug[:, cc, h * 65:(h + 1) * 65],
                                    tail_pts[i][:, half * 512:(half + 1) * 512],
                                    start=(cc == 0), stop=(cc == NCk - 1),
                                )
                        ptq.clear()
                    else:
                        for cc in range(NCk - 4, NCk):
                            emit_v(cc, ptq.pop(0))

                    # normalize: 1/denominator folded into the o^T copy
                    if g == NG - 1 and p == 1:
                        # tail: unblock the final out-projection st by st
                        rbs = []
                        for half in range(2):
                            h = 2 * p + half
                            rc = smalls.tile([1, 512], f32, tag="rc", name=f"rc{g}_{h}")
                            nc.vector.reciprocal(rc, ot[half][64:65, :])
                            rb = smalls.tile([64, 512], f32, tag="rb", name=f"rb{g}_{h}")
                            nc.gpsimd.partition_broadcast(rb, rc)
                            rbs.append(rb)
                        for q4 in range(4):
                            for half in range(2):
                                nc.vector.tensor_mul(
                                    oTn[p][half * 64:half * 64 + 64,
                                           g * 512 + q4 * 128:g * 512 + (q4 + 1) * 128],
                                    ot[half][0:64, q4 * 128:(q4 + 1) * 128],
                                    rbs[half][:, q4 * 128:(q4 + 1) * 128],
                                )
                    else:
                        for half in range(2):
                            h = 2 * p + half
                            rc = smalls.tile([1, 512], f32, tag="rc", name=f"rc{g}_{h}")
                            nc.vector.reciprocal(rc, ot[half][64:65, :])
                            rb = smalls.tile([64, 512], f32, tag="rb", name=f"rb{g}_{h}")
                            nc.gpsimd.partition_broadcast(rb, rc)
                            nc.vector.tensor_mul(
                                oTn[p][half * 64:half * 64 + 64, g * 512:(g + 1) * 512],
                                ot[half][0:64, :], rb[:],
                            )
                while wi < len(work):
                    work[wi][1]()
                    wi += 1
            for i, st in enumerate(range(4 * (NG - 1), 4 * NG)):
                state = {}
                for dg in range(D // 512):
                    if dg == 0:
                        ps = pp.tile([128, 512], f32, tag="pp", name=f"fps{st}_{dg}")
                    else:
                        ps = sp.tile([128, 1024], f32, tag="sps",
                                     name=f"fps{st}_{dg}")[:, 0:512]
                    state[dg] = ps
                    for kc in range(NT):
                        nc.tensor.matmul(
                            ps[:],
                            oTn[kc][:, st * 128:(st + 1) * 128],
                            wo_sb[:, kc, dg * 512:(dg + 1) * 512],
                            start=(kc == 0), stop=(kc == NT - 1),
                        )
                osb = outs.tile([128, D], bf16)
                if i % 2 == 0:
                    nc.scalar.activation(osb[:, 0:512], state[0][:],
                                         mybir.ActivationFunctionType.Copy)
                else:
                    nc.vector.tensor_copy(osb[:, 0:512], state[0][:])
                nc.sync.dma_start(out=OUT[st * 128:(st + 1) * 128, 0:512],
                                  in_=osb[:, 0:512])
                if i % 2 == 0:
                    nc.vector.tensor_copy(osb[:, 512:1024], state[1][:])
                else:
                    nc.scalar.activation(osb[:, 512:1024], state[1][:],
                                         mybir.ActivationFunctionType.Copy)
                nc.sync.dma_start(out=OUT[st * 128:(st + 1) * 128, 512:1024],
                                  in_=osb[:, 512:1024])

    nc.compile()
    return nc


def _get_nc():
    if "nc" not in _cached:
        _cached["nc"] = _build_nc()
    return _cached["nc"]


def _make_in_maps(inputs):
    queries = np.asarray(inputs["queries"], dtype=np.float32)
    keys = np.asarray(inputs["keys"], dtype=np.float32)
    values = np.asarray(inputs["values"], dtype=np.float32)
    Wq = np.asarray(inputs["Wq"], dtype=np.float32)
    Wk = np.asarray(inputs["Wk"], dtype=np.float32)
    Wv = np.asarray(inputs["Wv"], dtype=np.float32)
    Wo = np.asarray(inputs["Wo"], dtype=np.float32)
    bq = np.asarray(inputs["bq"], dtype=np.float32)
    bk = np.asarray(inputs["bk"], dtype=np.float32)
    bv = np.asarray(inputs["bv"], dtype=np.float32)
    mask = np.asarray(inputs["mask"])

    import ml_dtypes
    bf = ml_dtypes.bfloat16
    xqT = [np.ascontiguousarray(queries[b].T.astype(bf)) for b in range(B)]
    xkT = [np.ascontiguousarray(keys[b].T.astype(bf)) for b in range(B)]
    xvT = [np.ascontiguousarray(values[b].T.astype(bf)) for b in range(B)]
    maskT = [np.ascontiguousarray(mask[b, 0].T.astype(bf)) for b in range(B)]

    in_maps = []
    for c in range(NCORES):
        b = c // 4
        h0 = (c % 4) * HC
        sl = slice(h0 * DK, (h0 + HC) * DK)
        in_maps.append({
            "xqT": xqT[b], "xkT": xkT[b], "xvT": xvT[b],
            "wq": np.ascontiguousarray(Wq[:, sl].astype(bf)),
            "wk": np.ascontiguousarray(Wk[:, sl].astype(bf)),
            "wv": np.ascontiguousarray(Wv[:, sl].astype(bf)),
            "wo": np.ascontiguousarray(Wo[sl, :].astype(bf)),
            "bqc": np.ascontiguousarray(bq[sl].reshape(NT, 128).T.astype(np.float32)),
            "bkc": np.ascontiguousarray(bk[sl].reshape(NT, 128).T.astype(np.float32)),
            "bv": np.ascontiguousarray(bv[sl].reshape(1, NC_)),
            "maskT": maskT[b],
        })
    return in_maps


def _combine(results, bo):
    out = np.empty((B, S, D), dtype=np.float32)
    for b in range(B):
        acc = results[4 * b]["out"].astype(np.float32)
        for c in range(4 * b + 1, 4 * b + 4):
            acc = acc + results[c]["out"].astype(np.float32)
        out[b] = acc + bo[None, :]
    return out


def kernel(queries, keys, values, Wq, bq, Wk, bk, Wv, bv, Wo, bo, mask):
    from concourse.bass_utils import run_bass_kernel_spmd

    nc = _get_nc()
    in_maps = _make_in_maps(dict(
        queries=queries, keys=keys, values=values, Wq=Wq, Wk=Wk, Wv=Wv, Wo=Wo,
        bq=bq, bk=bk, bv=bv, mask=mask))
    res = run_bass_kernel_spmd(nc, in_maps, list(range(NCORES)))
    return _combine(res.results, np.asarray(bo, dtype=np.float32))


# revision 39
# speedup vs baseline: 1.0275x; 1.0275x over previous
"""MultiHeadAttention Trainium2 kernel.

Sharding: B=2 batches x H=16 heads = 32 (b,h) pairs -> 4 heads per core.
Cores 0-3 handle batch 0 (heads 4c..4c+3), cores 4-7 batch 1.
Each core computes q/k/v projections for its head slice, transposed-scores
attention, and a partial output projection (sum over its heads of
o_h @ Wo[h-slice]).  Host sums the 4 bf16 partials per batch and adds bo.

All-bf16 datapath (fp8 operand storage was measured to break the 2e-2
tolerance: qk8 5.5e-2, P8+v8+o8 4.2e-2).  Biases fold into the PSUM->SBUF
copies (per-partition tensor_scalar_add for q/k, broadcast add for v), so
the PE runs no bias rows.  Softmax skips max-subtraction (scores ~ N(0,1));
the mask multiply runs on DVE (bf16 2x) against a per-group streamed maskT;
denominators come from a ones-column appended to V; and 1/denominator is
folded into the PSUM->SBUF copy of o^T.

Schedule: the attention c-loop is Act-bound (one [128,1024] exp per chunk,
1038ns each), so every other engine hides under it.  The loop runs per
(sq-group, head-pair) pass so the P@V accumulators need only 2 PSUM banks,
leaving a dedicated 2-bank projection pool; projection and output-
projection matmuls drip into the loop a few per chunk from a deadline-
sorted worklist, keeping the PE dense without stalling the exp-paced
scores-PSUM rotation (2 x [128,1024]).  P@V trails the exp/mask pipeline
by 4 chunks to decouple DVE jitter.  A dummy-matmul chain warms the PE
p-state during the initial DMA fill; the first k/q projections stream in
half-tensor DMA pieces; the tail reorders the last P@V half-pairs and
splits the final normalize per 128 columns so the last output projections
start as early as possible.
"""

import sys

sys.path.insert(0, '/opt/trn_rl_repo')

import numpy as np

B, S, D = 2, 2048, 1024
H = 16
DK = 64
HC = 4            # heads per core
NC_ = HC * DK     # 256 projected dims per core
NT = NC_ // 128   # head-pair tiles per core
NCORES = 8

_cached = {}


def _build_nc():
    import concourse.bacc as bacc
    import concourse.mybir as mybir
    from concourse.tile import TileContext

    f32 = mybir.dt.float32
    bf16 = mybir.dt.bfloat16
    Exp = mybir.ActivationFunctionType.Exp
    Identity = mybir.ActivationFunctionType.Identity

    nc = bacc.Bacc()

    XQT = nc.declare_dram_parameter("xqT", [D, S], bf16, isOutput=False)
    XKT = nc.declare_dram_parameter("xkT", [D, S], bf16, isOutput=False)
    XVT = nc.declare_dram_parameter("xvT", [D, S], bf16, isOutput=False)
    WQ = nc.declare_dram_parameter("wq", [D, NC_], bf16, isOutput=False)
    WK = nc.declare_dram_parameter("wk", [D, NC_], bf16, isOutput=False)
    WV = nc.declare_dram_parameter("wv", [D, NC_], bf16, isOutput=False)
    WO = nc.declare_dram_parameter("wo", [NC_, D], bf16, isOutput=False)
    BQC = nc.declare_dram_parameter("bqc", [128, 2], f32, isOutput=False)
    BKC = nc.declare_dram_parameter("bkc", [128, 2], f32, isOutput=False)
    BV = nc.declare_dram_parameter("bv", [1, NC_], f32, isOutput=False)
    MT = nc.declare_dram_parameter("maskT", [S, S], bf16, isOutput=False)
    OUT = nc.declare_dram_parameter("out", [S, D], bf16, isOutput=True)

    NDC = D // 128           # 8 d chunks
    NG = S // 512            # 4 sq groups / k s-groups
    NCk = S // 128           # 16 sk chunks

    with TileContext(nc) as tc:
        import contextlib
        ctx = contextlib.ExitStack()
        with ctx:
            consts = ctx.enter_context(tc.tile_pool(name="consts", bufs=1))
            xts = ctx.enter_context(tc.tile_pool(name="xts", bufs=1))
            pts = ctx.enter_context(tc.tile_pool(name="pts", bufs=3))
            smalls = ctx.enter_context(tc.tile_pool(name="smalls", bufs=3))
            outs = ctx.enter_context(tc.tile_pool(name="outs", bufs=3))
            mts = ctx.enter_context(tc.tile_pool(name="mts", bufs=2))
            sp = ctx.enter_context(tc.tile_pool(name="sp", bufs=2, space="PSUM"))
            pp = ctx.enter_context(tc.tile_pool(name="pp", bufs=2, space="PSUM"))
            op = ctx.enter_context(tc.tile_pool(name="op", bufs=1, space="PSUM"))

            # ---- constants (DMA order matters: first-needed first) ----
            wk_sb = consts.tile([128, NDC, NC_], bf16)
            wq_sb = consts.tile([128, NDC, NC_], bf16)
            wv_sb = consts.tile([128, NDC, NC_], bf16)
            wo_sb = consts.tile([128, NT, D], bf16)
            bqc_sb = consts.tile([128, NT], f32)
            bkc_sb = consts.tile([128, NT], f32)
            bv_row = consts.tile([1, NC_], f32)

            xtiles = {}

            def emit_x_dma(which, g, bufs=4, split=False):
                X = {"k": XKT, "q": XQT, "v": XVT}[which]
                xg = xts.tile([128, NDC, 512], bf16, tag=f"x{which}",
                              name=f"x{which}{g}", bufs=bufs)
                if split:
                    for hh in range(2):
                        nc.sync.dma_start(
                            out=xg[:, 4 * hh:4 * (hh + 1), :],
                            in_=X[4 * hh * 128:4 * (hh + 1) * 128,
                                  g * 512:(g + 1) * 512]
                            .rearrange("(c p) n -> p c n", p=128))
                else:
                    nc.sync.dma_start(
                        out=xg,
                        in_=X[:, g * 512:(g + 1) * 512].rearrange("(c p) n -> p c n", p=128))
                xtiles[(which, g)] = xg

            mtiles = {}

            def emit_mask_dma(g, piece=None, c0=None, c1=None):
                # mask columns for sq-group g; [128, NCk, 512] per group.
                if g not in mtiles:
                    mtiles[g] = mts.tile([128, NCk, 512], bf16, tag="mt",
                                         name=f"mt{g}", bufs=2)
                if c0 is None:
                    if piece is None:
                        c0, c1 = 0, NCk
                    else:
                        c0, c1 = 4 * piece, 4 * piece + 4
                nc.sync.dma_start(
                    out=mtiles[g][:, c0:c1, :],
                    in_=MT[c0 * 128:c1 * 128, g * 512:(g + 1) * 512]
                    .rearrange("(c p) s -> p c s", p=128))

            warm = consts.tile([1, 512], bf16)
            nc.vector.memset(warm, 1.0)
            for wu in range(2):
                wps = pp.tile([128, 512], f32, tag="pp", name=f"warmps{wu}")
                for _ in range(3):
                    nc.tensor.matmul(wps[0:1, :], warm[0:1, 0:1], warm[0:1, :],
                                     start=True, stop=True)

            xk0 = xts.tile([128, NDC, 512], bf16, tag="xk", name="xk0", bufs=4)
            xq0 = xts.tile([128, NDC, 512], bf16, tag="xq", name="xq0", bufs=2)
            xtiles[("k", 0)] = xk0
            xtiles[("q", 0)] = xq0
            nc.sync.dma_start(out=wk_sb[:, 0:4, :],
                              in_=WK[0:512].rearrange("(c p) n -> p c n", p=128))
            nc.sync.dma_start(out=xk0[:, 0:4, :],
                              in_=XKT[0:512, 0:512].rearrange("(c p) n -> p c n", p=128))
            nc.sync.dma_start(out=wq_sb[:, 0:4, :],
                              in_=WQ[0:512].rearrange("(c p) n -> p c n", p=128))
            nc.sync.dma_start(out=xq0[:, 0:4, :],
                              in_=XQT[0:512, 0:512].rearrange("(c p) n -> p c n", p=128))
            nc.sync.dma_start(out=wk_sb[:, 4:8, :],
                              in_=WK[512:1024].rearrange("(c p) n -> p c n", p=128))
            nc.sync.dma_start(out=xk0[:, 4:8, :],
                              in_=XKT[512:1024, 0:512].rearrange("(c p) n -> p c n", p=128))
            nc.sync.dma_start(out=bkc_sb, in_=BKC[:])
            nc.sync.dma_start(out=wq_sb[:, 4:8, :],
                              in_=WQ[512:1024].rearrange("(c p) n -> p c n", p=128))
            nc.sync.dma_start(out=xq0[:, 4:8, :],
                              in_=XQT[512:1024, 0:512].rearrange("(c p) n -> p c n", p=128))
            nc.sync.dma_start(out=bqc_sb, in_=BQC[:])
            nc.sync.dma_start(out=wv_sb, in_=WV[:].rearrange("(c p) n -> p c n", p=128))
            emit_x_dma("v", 0, bufs=3)
            emit_mask_dma(0, piece=0)
            nc.sync.dma_start(out=bv_row, in_=BV[:])
            emit_mask_dma(0, piece=1)
            emit_x_dma("k", 1)
            emit_mask_dma(0, piece=2)
            emit_x_dma("v", 1, bufs=3)
            emit_mask_dma(0, piece=3)
            emit_x_dma("q", 1, bufs=2)
            emit_x_dma("k", 2)
            emit_x_dma("v", 2, bufs=3)
            emit_x_dma("k", 3)
            emit_x_dma("v", 3, bufs=3)
            nc.sync.dma_start(out=wo_sb, in_=WO[:].rearrange("(c p) n -> p c n", p=128))
            bv_bc = consts.tile([128, NC_], f32)
            nc.gpsimd.partition_broadcast(bv_bc, bv_row)

            qT = [consts.tile([128, S], bf16, tag=f"qT{i}", name=f"qT{i}") for i in range(NT)]
            kT = [consts.tile([128, S], bf16, tag=f"kT{i}", name=f"kT{i}") for i in range(NT)]
            v_aug = consts.tile([128, NCk, HC * 65], bf16)
            nc.gpsimd.memset(v_aug, 1.0)
            oTn = [consts.tile([128, S], bf16, tag=f"oTn{i}", name=f"oTn{i}") for i in range(NT)]

            # ---- worklist quanta (deadline, closure) ----
            def kq_nt_quanta(g, which, nt, dl):
                """4 quanta of one n-tile of a k/q projection, deadlines dl-3..dl."""
                W, BC, T = ((wk_sb, bkc_sb, kT) if which == "k"
                            else (wq_sb, bqc_sb, qT))
                state = {}

                def start(state=state, nt=nt, g=g, which=which):
                    state["ps"] = pp.tile([128, 512], f32, tag="pp",
                                          name=f"{which}ps{g}_{nt}")

                def mms(dc, state=state, nt=nt, g=g, W=W, which=which):
                    xg = xtiles[(which, g)]
                    nc.tensor.matmul(
                        state["ps"][:],
                        W[:, dc, nt * 128:(nt + 1) * 128],
                        xg[:, dc, :],
                        start=(dc == 0), stop=(dc == NDC - 1),
                    )

                def fin(state=state, nt=nt, g=g, T=T, BC=BC):
                    nc.vector.tensor_scalar_add(
                        T[nt][:, g * 512:(g + 1) * 512], state["ps"][:],
                        BC[:, nt:nt + 1])

                return [(dl - 3, lambda s=start, m=mms: (s(), m(0), m(1))),
                        (dl - 2, lambda m=mms: (m(2), m(3))),
                        (dl - 1, lambda m=mms: (m(4), m(5))),
                        (dl, lambda m=mms, f=fin: (m(6), m(7), f()))]

            def v_sl_quanta(g, sl, dl):
                st = 4 * g + sl
                state = {}

                def start(state=state, st=st):
                    state["ps"] = pp.tile([128, 512], f32, tag="pp", name=f"vps{st}")

                def mms(dc0, state=state, sl=sl, g=g):
                    xgv = xtiles[("v", g)]
                    for dc in range(dc0, dc0 + 4):
                        nc.tensor.matmul(
                            state["ps"][:, 0:NC_],
                            xgv[:, dc, sl * 128:(sl + 1) * 128],
                            wv_sb[:, dc, :],
                            start=(dc == 0), stop=(dc == NDC - 1),
                        )

                def fin(state=state, st=st):
                    for h in range(HC):
                        nc.vector.tensor_add(
                            out=v_aug[:, st, h * 65:h * 65 + 64],
                            in0=state["ps"][:, h * 64:(h + 1) * 64],
                            in1=bv_bc[:, h * 64:(h + 1) * 64],
                        )

                return [(dl - 1, lambda s=start, m=mms: (s(), m(0))),
                        (dl, lambda m=mms, f=fin: (m(4), f()))]

            def kq_nt_quanta_fine(g, which, nt, dl0):
                """8 single-matmul quanta (contiguous deadlines) + fin."""
                W, BC, T = ((wk_sb, bkc_sb, kT) if which == "k"
                            else (wq_sb, bqc_sb, qT))
                state = {}

                def mm(dc, state=state, nt=nt, g=g, W=W, which=which):
                    if dc == 0:
                        state["ps"] = pp.tile([128, 512], f32, tag="pp",
                                              name=f"{which}ps{g}_{nt}")
                    xg = xtiles[(which, g)]
                    nc.tensor.matmul(
                        state["ps"][:],
                        W[:, dc, nt * 128:(nt + 1) * 128],
                        xg[:, dc, :],
                        start=(dc == 0), stop=(dc == NDC - 1),
                    )

                def fin(state=state, nt=nt, g=g, T=T, BC=BC):
                    nc.vector.tensor_scalar_add(
                        T[nt][:, g * 512:(g + 1) * 512], state["ps"][:],
                        BC[:, nt:nt + 1])

                out = [(dl0 + j, lambda mm=mm, j=j: mm(j)) for j in range(NDC - 1)]
                out.append((dl0 + NDC - 1, lambda mm=mm, f=fin: (mm(NDC - 1), f())))
                return out

            def outproj_quanta_fine(st, dl):
                """4 single-matmul quanta + two copy/DMA finishers."""
                state = {}
                out = []
                for j, (dg, kc) in enumerate([(0, 0), (0, 1), (1, 0), (1, 1)]):
                    def mk(dg=dg, kc=kc, state=state, st=st):
                        if kc == 0:
                            state[dg] = pp.tile([128, 512], f32, tag="pp",
                                                name=f"fps{st}_{dg}")
                        nc.tensor.matmul(
                            state[dg][:],
                            oTn[kc][:, st * 128:(st + 1) * 128],
                            wo_sb[:, kc, dg * 512:(dg + 1) * 512],
                            start=(kc == 0), stop=(kc == NT - 1),
                        )
                    out.append((dl + j, mk))

                def fin0(state=state, st=st):
                    osb = outs.tile([128, D], bf16)
                    state["osb"] = osb
                    nc.vector.tensor_copy(osb[:, 0:512], state[0][:])

                def fin1(state=state, st=st):
                    osb = state["osb"]
                    nc.vector.tensor_copy(osb[:, 512:1024], state[1][:])
                    nc.sync.dma_start(out=OUT[st * 128:(st + 1) * 128, :], in_=osb[:])
                out.append((dl + 4, fin0))
                out.append((dl + 5, fin1))
                return out

            def outproj_quanta(st, dl):
                state = {}
                out = []
                for dg in range(D // 512):
                    def mk(dg=dg, state=state, st=st):
                        ps = pp.tile([128, 512], f32, tag="pp", name=f"fps{st}_{dg}")
                        state[dg] = ps
                        for kc in range(NT):
                            nc.tensor.matmul(
                                ps[:],
                                oTn[kc][:, st * 128:(st + 1) * 128],
                                wo_sb[:, kc, dg * 512:(dg + 1) * 512],
                                start=(kc == 0), stop=(kc == NT - 1),
                            )
                    out.append((dl + dg, mk))

                def fin0(state=state, st=st):
                    osb = outs.tile([128, D], bf16)
                    state["osb"] = osb
                    nc.vector.tensor_copy(osb[:, 0:512], state[0][:])

                def fin1(state=state, st=st):
                    osb = state["osb"]
                    nc.vector.tensor_copy(osb[:, 512:1024], state[1][:])
                    nc.sync.dma_start(out=OUT[st * 128:(st + 1) * 128, :], in_=osb[:])
                out.append((dl + 2, fin0))
                out.append((dl + 3, fin1))
                return out

            # ---- prologue: just the first head-pair's k/q projections ----
            for _, q in kq_nt_quanta(0, "k", 0, 0):
                q()
            for _, q in kq_nt_quanta(0, "q", 0, 0):
                q()

            # ---- attention: per (sq-group, head-pair) pass ----
            for g in range(NG):
                if g + 1 < NG:
                    emit_mask_dma(g + 1)
                work = []
                if g == 0:
                    work += kq_nt_quanta(0, "q", 1, 0)
                    work += kq_nt_quanta(0, "k", 1, 1)
                    for sl in range(4):
                        work += v_sl_quanta(0, sl, sl + 2)
                    for s2 in range(1, NG):
                        work += kq_nt_quanta(s2, "k", 0, 4 * s2 - 2)
                        for sl in range(4):
                            work += v_sl_quanta(s2, sl, 4 * s2 + sl)
                    for s2 in range(1, NG):
                        work += kq_nt_quanta(s2, "k", 1, 14 + 4 * s2)
                    work += kq_nt_quanta(1, "q", 0, 26)
                    work += kq_nt_quanta(1, "q", 1, 30)
                else:
                    if g < NG - 1:
                        emit_x_dma("q", g + 1, bufs=2)
                        work += kq_nt_quanta_fine(g + 1, "q", 0, 0)
                        work += kq_nt_quanta_fine(g + 1, "q", 1, 8)
                    sts = {1: [0, 1], 2: [2, 3, 4, 5], 3: [6, 7, 8, 9, 10, 11]}[g]
                    base = 16 if g < NG - 1 else 0
                    span = max(1, (31 - base) // len(sts))
                    for i, st in enumerate(sts):
                        work += outproj_quanta_fine(st, base + span * i)
                work.sort(key=lambda x: x[0])
                wi = 0

                def drip(t, work=work):
                    nonlocal wi
                    rem = len(work) - wi
                    if rem <= 0:
                        return
                    iters_left = max(1, 32 - t)
                    n = max(0, -(-rem // iters_left))
                    while wi < len(work) and (work[wi][0] <= t + 2 or n > 0):
                        work[wi][1]()
                        wi += 1
                        n -= 1

                for p in range(2):
                    ot = [op.tile([65, 512], f32, tag=f"ot{half}",
                                  name=f"ot{p}_{half}_{g}")
                          for half in range(2)]

                    def emit_v(c, pt, p=p, ot=ot):
                        for half in range(2):
                            h = 2 * p + half
                            nc.tensor.matmul(
                                ot[half][:],
                                v_aug[:, c, h * 65:(h + 1) * 65],
                                pt[:, half * 512:(half + 1) * 512],
                                start=(c == 0), stop=(c == NCk - 1),
                            )

                    ptq = []
                    for c in range(NCk):
                        sps = sp.tile([128, 1024], f32, tag="sps",
                                      name=f"sps{g}_{p}_{c}")
                        for half in range(2):
                            nc.tensor.matmul(
                                sps[:, half * 512:(half + 1) * 512],
                                kT[p][half * 64:half * 64 + 64, c * 128:(c + 1) * 128],
                                qT[p][half * 64:half * 64 + 64, g * 512:(g + 1) * 512],
                                start=True, stop=True,
                            )
                        pt = pts.tile([128, 1024], bf16, tag="pt",
                                      name=f"pt{g}_{p}_{c}", bufs=6)
                        nc.scalar.activation(pt[:], sps[:], Exp, scale=0.125)
                        for half in range(2):
                            nc.vector.tensor_mul(
                                pt[:, half * 512:(half + 1) * 512],
                                pt[:, half * 512:(half + 1) * 512],
                                mtiles[g][:, c, :])
                        ptq.append(pt)
                        if len(ptq) > 4:
                            emit_v(c - 4, ptq.pop(0))
                        drip(p * NCk + c)
                    if g == NG - 1 and p == 1:
                        # half-major: all h2 PVs then all h3, so the h2
                        # denominator reciprocal starts earlier
                        tail_pts = list(ptq)
                        for half in range(2):
                            for i, cc in enumerate(range(NCk - 4, NCk)):
                                h = 2 * p + half
                                nc.tensor.matmul(
                                    ot[half][:],
                                    v_aug[:, cc, h * 65:(h + 1) * 65],
                                    tail_pts[i][:, half * 512:(half + 1) * 512],
                                    start=(cc == 0), stop=(cc == NCk - 1),
                                )
                        ptq.clear()
                    else:
                        for cc in range(NCk - 4, NCk):
                            emit_v(cc, ptq.pop(0))

                    # normalize: 1/denominator folded into the o^T copy
                    if g == NG - 1 and p == 1:
                        # tail: unblock the final out-projection st by st
                        rbs = []
                        for half in range(2):
                            h = 2 * p + half
                            rc = smalls.tile([1, 512], f32, tag="rc", name=f"rc{g}_{h}")
                            nc.vector.reciprocal(rc, ot[half][64:65, :])
                            rb = smalls.tile([64, 512], f32, tag="rb", name=f"rb{g}_{h}")
                            nc.gpsimd.partition_broadcast(rb, rc)
                            rbs.append(rb)
                        for q4 in range(4):
                            for half in range(2):
                                nc.vector.tensor_mul(
                                    oTn[p][half * 64:half * 64 + 64,
                                           g * 512 + q4 * 128:g * 512 + (q4 + 1) * 128],
                                    ot[half][0:64, q4 * 128:(q4 + 1) * 128],
                                    rbs[half][:, q4 * 128:(q4 + 1) * 128],
                                )
                    else:
                        for half in range(2):
                            h = 2 * p + half
                            rc = smalls.tile([1, 512], f32, tag="rc", name=f"rc{g}_{h}")
                            nc.vector.reciprocal(rc, ot[half][64:65, :])
                            rb = smalls.tile([64, 512], f32, tag="rb", name=f"rb{g}_{h}")
                            nc.gpsimd.partition_broadcast(rb, rc)
                            nc.vector.tensor_mul(
                                oTn[p][half * 64:half * 64 + 64, g * 512:(g + 1) * 512],
                                ot[half][0:64, :], rb[:],
                            )
                while wi < len(work):
                    work[wi][1]()
                    wi += 1
            for i, st in enumerate(range(4 * (NG - 1), 4 * NG)):
                state = {}
                for dg in range(D // 512):
                    if dg == 0:
                        ps = pp.tile([128, 512], f32, tag="pp", name=f"fps{st}_{dg}")
                    else:
                        ps = sp.tile([128, 1024], f32, tag="sps",
                                     name=f"fps{st}_{dg}")[:, 0:512]
                    state[dg] = ps
                    for kc in range(NT):
                        nc.tensor.matmul(
                            ps[:],
                            oTn[kc][:, st * 128:(st + 1) * 128],
                            wo_sb[:, kc, dg * 512:(dg + 1) * 512],
                            start=(kc == 0), stop=(kc == NT - 1),
                        )
                osb = outs.tile([128, D], bf16)
                if i % 2 == 0:
                    nc.scalar.activation(osb[:, 0:512], state[0][:],
                                         mybir.ActivationFunctionType.Copy)
                else:
                    nc.vector.tensor_copy(osb[:, 0:512], state[0][:])
                nc.sync.dma_start(out=OUT[st * 128:(st + 1) * 128, 0:512],
                                  in_=osb[:, 0:512])
                if i % 2 == 0:
                    nc.vector.tensor_copy(osb[:, 512:1024], state[1][:])
                else:
                    nc.scalar.activation(osb[:, 512:1024], state[1][:],
                                         mybir.ActivationFunctionType.Copy)
                nc.sync.dma_start(out=OUT[st * 128:(st + 1) * 128, 512:1024],
                                  in_=osb[:, 512:1024])

    nc.compile()
    return nc


def _get_nc():
    if "nc" not in _cached:
        _cached["nc"] = _build_nc()
    return _cached["nc"]


def _make_in_maps(inputs):
    queries = np.asarray(inputs["queries"], dtype=np.float32)
    keys = np.asarray(inputs["keys"], dtype=np.float32)
    values = np.asarray(inputs["values"], dtype=np.float32)
    Wq = np.asarray(inputs["Wq"], dtype=np.float32)
    Wk = np.asarray(inputs["Wk"], dtype=np.float32)
    Wv = np.asarray(inputs["Wv"], dtype=np.float32)
    Wo = np.asarray(inputs["Wo"], dtype=np.float32)
    bq = np.asarray(inputs["bq"], dtype=np.float32)
    bk = np.asarray(inputs["bk"], dtype=np.float32)
    bv = np.asarray(inputs["bv"], dtype=np.float32)
    mask = np.asarray(inputs["mask"])

    import ml_dtypes
    bf = ml_dtypes.bfloat16
    xqT = [np.ascontiguousarray(queries[b].T.astype(bf)) for b in range(B)]
    xkT = [np.ascontiguousarray(keys[b].T.astype(bf)) for b in range(B)]
    xvT = [np.ascontiguousarray(values[b].T.astype(bf)) for b in range(B)]
    maskT = [np.ascontiguousarray(mask[b, 0].T.astype(bf)) for b in range(B)]

    in_maps = []
    for c in range(NCORES):
        b = c // 4
        h0 = (c % 4) * HC
        sl = slice(h0 * DK, (h0 + HC) * DK)
        in_maps.append({
            "xqT": xqT[b], "xkT": xkT[b], "xvT": xvT[b],
            "wq": np.ascontiguousarray(Wq[:, sl].astype(bf)),
            "wk": np.ascontiguousarray(Wk[:, sl].astype(bf)),
            "wv": np.ascontiguousarray(Wv[:, sl].astype(bf)),
            "wo": np.ascontiguousarray(Wo[sl, :].astype(bf)),
            "bqc": np.ascontiguousarray(bq[sl].reshape(NT, 128).T.astype(np.float32)),
            "bkc": np.ascontiguousarray(bk[sl].reshape(NT, 128).T.astype(np.float32)),
            "bv": np.ascontiguousarray(bv[sl].reshape(1, NC_)),
            "maskT": maskT[b],
        })
    return in_maps


def _combine(results, bo):
    out = np.empty((B, S, D), dtype=np.float32)
    for b in range(B):
        acc = results[4 * b]["out"].astype(np.float32)
        for c in range(4 * b + 1, 4 * b + 4):
            acc = acc + results[c]["out"].astype(np.float32)
        out[b] = acc + bo[None, :]
    return out


def kernel(queries, keys, values, Wq, bq, Wk, bk, Wv, bv, Wo, bo, mask):
    from concourse.bass_utils import run_bass_kernel_spmd

    nc = _get_nc()
    in_maps = _make_in_maps(dict(
        queries=queries, keys=keys, values=values, Wq=Wq, Wk=Wk, Wv=Wv, Wo=Wo,
        bq=bq, bk=bk, bv=bv, mask=mask))
    res = run_bass_kernel_spmd(nc, in_maps, list(range(NCORES)))
    return _combine(res.results, np.asarray(bo, dtype=np.float32))
